# revision 6
# baseline (speedup 1.0000x reference)
"""AttentionLSTM Trainium2 kernel, 8-core SPMD.

Strategy: tensor-parallel over the 4H gate dimension. Core k owns H-slice
hd in [128k, 128(k+1)) of each of the four gates (512 act columns, laid out
[i_k | f_k | o_k | g_k]) and the matching slices of the c/h state.

Per timestep, one AllGather carries [partial_scores(16) | hT-slice(128)]
per batch row (bf16): the gathered hT blocks form exactly the lhsT k-tiles
for h@Wh, and the 8 partial-score blocks sum to the full attention scores
on every core.  The attention's contribution to the gate pre-activations
is computed without materializing attn: the prologue builds
B[n,p,j] = sum_h Af[n,h,p] * Wattn[h,j]  (j-sharded), and each step does
act += sum_p w[n,p] * B[n,p,j] as 16 diagonal matmuls diag(w_p) @ B_p
accumulated into the same PSUM bank as x@Wx (prefetched during the
AllGather window) and h@Wh.
"""
import numpy as np

from concourse import bacc, tile
from concourse import mybir
from concourse.bass_utils import run_bass_kernel_spmd

N, T, D, H = 128, 64, 1024, 1024
P16 = 16          # attention positions (4x4)
NC = 8            # cores
HS = H // NC      # 128, per-core H slice
JS = 4 * HS       # 512, per-core act columns
KT = D // 128     # 8 k-tiles

F32 = mybir.dt.float32
F32R = mybir.dt.float32r
BF16 = mybir.dt.bfloat16
RG = [list(range(NC))]

_nc_cache = None


def _build():
    nc = bacc.Bacc("TRN2", target_bir_lowering=False, debug=False, num_devices=NC)

    xT_d = nc.dram_tensor("xT", [T, 128, KT, N], F32, kind="ExternalInput").ap()
    wx_d = nc.dram_tensor("wx", [D, JS], F32, kind="ExternalInput").ap()
    wh_d = nc.dram_tensor("wh", [H, JS], F32, kind="ExternalInput").ap()
    wattn_d = nc.dram_tensor("wattn", [H, JS], F32, kind="ExternalInput").ap()
    b_d = nc.dram_tensor("bvec", [1, JS], F32, kind="ExternalInput").ap()
    ident_d = nc.dram_tensor("ident", [128, 128], F32, kind="ExternalInput").ap()
    afn_d = nc.dram_tensor("afn", [N, P16, HS], F32, kind="ExternalInput").ap()
    afT_d = nc.dram_tensor("afT", [H, P16, N], F32, kind="ExternalInput").ap()
    out_d = nc.dram_tensor("out", [T, N, HS], BF16, kind="ExternalOutput").ap()

    with tile.TileContext(nc) as tc:
        with (
            tc.tile_pool(name="const", bufs=1) as cp,
            tc.tile_pool(name="state", bufs=2) as sp,
            tc.tile_pool(name="work", bufs=2) as wp,
            tc.tile_pool(name="xpool", bufs=3) as xp,
            tc.tile_pool(name="psum", bufs=2, space="PSUM") as pp,
            tc.tile_pool(name="tpsum", bufs=2, space="PSUM") as tp,
            tc.tile_pool(name="dram", bufs=2, space="DRAM") as dp,
        ):
            # ---------------- constants ----------------
            wx_b = cp.tile([128, KT, JS], BF16, name="wx_b")
            b_b = cp.tile([1, JS], BF16, name="b_b")
            ones_b = cp.tile([1, 128], BF16, name="ones_b")
            nc.vector.memset(ones_b[:], 1.0)
            ident = cp.tile([128, 128], F32, name="ident")
            nc.sync.dma_start(out=ident[:], in_=ident_d[:])
            ident_b = cp.tile([128, 128], BF16, name="ident_b")
            nc.vector.tensor_copy(ident_b[:], ident[:])
            warm0 = cp.tile([128, 2048], BF16, name="warm0")
            nc.vector.memset(warm0[:], 1.0)
            ident_rep = cp.tile([128, P16, 128], BF16, name="ident_rep")
            nc.vector.tensor_copy(
                ident_rep[:], ident_b[:].unsqueeze(1).broadcast_to([128, P16, 128])
            )
            wh_b = cp.tile([128, KT, JS], BF16, name="wh_b")
            afn_b = cp.tile([128, P16, HS], BF16, name="afn_b")
            B_s = cp.tile([128, P16, JS], BF16, name="B_s")

            # ---------------- prologue ----------------
            h_cur = sp.tile([128, HS], BF16, name="h", tag="h")
            c_cur = sp.tile([128, HS], F32, name="c", tag="c")
            with (
                tc.tile_pool(name="prol", bufs=1) as pr,
                tc.tile_pool(name="prolp", bufs=2, space="PSUM") as prp,
            ):
                wat_b = pr.tile([128, KT, JS], BF16, name="wat_b")
                afT_b = pr.tile([128, KT, P16, N], BF16, name="afT_b")
                for wsrc, wdst in ((wx_d, wx_b), (wh_d, wh_b), (wattn_d, wat_b)):
                    wst = pr.tile([128, KT, JS], F32, name="wst", tag="wst", bufs=2)
                    nc.sync.dma_start(out=wst[:], in_=wsrc[:].rearrange("(kk p) j -> p kk j", kk=KT))
                    nc.vector.tensor_copy(wdst[:], wst[:])
                b_st = pr.tile([1, JS], F32, name="b_st")
                nc.sync.dma_start(out=b_st[:], in_=b_d[:])
                nc.vector.tensor_copy(b_b[:], b_st[:])

                afn_st = pr.tile([128, P16, HS], F32, name="afn_st")
                nc.sync.dma_start(out=afn_st[:], in_=afn_d[:])
                nc.vector.tensor_copy(afn_b[:], afn_st[:])
                # h0 = mean over p of Af (slice), c0 = h0
                h0r = pr.tile([128, HS], F32, name="h0r")
                nc.vector.tensor_reduce(
                    h0r[:], afn_st[:].rearrange("n p hd -> n hd p"),
                    mybir.AxisListType.X, mybir.AluOpType.add,
                )
                nc.scalar.mul(out=h_cur[:], in_=h0r[:], mul=1.0 / P16)
                nc.scalar.mul(out=c_cur[:], in_=h0r[:], mul=1.0 / P16)

                # B[n,p,j] = sum_h Af[n,h,p] Wattn[h,j]
                afT_src = afT_d[:].rearrange("(kk p) q n -> p kk q n", kk=KT)
                for kk in range(KT):
                    afT_st = pr.tile([128, P16, N], F32, name="afT_st", tag="afT_st", bufs=2)
                    nc.sync.dma_start(out=afT_st[:], in_=afT_src[:, kk])
                    nc.vector.tensor_copy(afT_b[:, kk], afT_st[:])
                for p in range(P16):
                    bp = prp.tile([128, JS], F32, name="bp", tag="bp")
                    for kk in range(KT):
                        nc.tensor.matmul(
                            out=bp[:],
                            lhsT=afT_b[:, kk, p, :],
                            rhs=wat_b[:, kk, :],
                            start=(kk == 0), stop=(kk == KT - 1),
                        )
                    nc.scalar.copy(out=B_s[:, p, :], in_=bp[:])

            # ---------------- recurrence ----------------
            act_prev = None

            def xwx_prefetch(t):
                """x_t @ Wx + b into a fresh act psum bank (during AG window)."""
                xt = xp.tile([128, KT, N], F32, name="xt", tag="xt")
                nc.sync.dma_start(out=xt[:], in_=xT_d[t])
                xt_b = xp.tile([128, KT, N], BF16, name="xt_b", tag="xt_b")
                nc.scalar.copy(out=xt_b[:], in_=xt[:])
                act = pp.tile([128, JS], F32, name="act", tag="act")
                for kk in range(KT):
                    nc.tensor.matmul(
                        out=act[:],
                        lhsT=xt_b[:, kk, :],
                        rhs=wx_b[:, kk, :],
                        start=(kk == 0), stop=False,
                    )
                nc.tensor.matmul(
                    out=act[:], lhsT=ones_b[:], rhs=b_b[:],
                    start=False, stop=False,
                )
                return act

            act_prev = xwx_prefetch(0)

            for t in range(T):
                PW = P16 + 128  # payload width (bf16 cols): 16 pscores + 128 hT
                # ---- payload: [pscores bf16 | hT] in one SBUF tile ----
                bin_ = dp.tile([128, PW], BF16, name="bin", tag="bin")
                bout = dp.tile([NC * 128, PW], BF16, addr_space="Shared",
                               name="bout", tag="bout")
                payload = wp.tile([128, PW], BF16, name="payload", tag="payload")
                hTp = tp.tile([128, 128], BF16, name="hTp", tag="hTp")
                nc.tensor.transpose(hTp[:], h_cur[:], ident_b[:])
                nc.scalar.copy(out=payload[:, P16:], in_=hTp[:])
                prod = wp.tile([128, P16, HS], BF16, name="prod", tag="prod")
                nc.vector.tensor_mul(
                    prod[:], afn_b[:],
                    h_cur[:].unsqueeze(1).broadcast_to([128, P16, HS]),
                )
                with nc.allow_low_precision("pscores bf16 accumulate, 128 terms"):
                    nc.vector.tensor_reduce(
                        payload[:, 0:P16], prod[:],
                        mybir.AxisListType.X, mybir.AluOpType.add,
                    )
                nc.sync.dma_start(out=bin_[0:64, :], in_=payload[0:64, :])
                nc.scalar.dma_start(out=bin_[64:128, :], in_=payload[64:128, :])

                # ---- AllGather + gather readback split across both queues ----
                nc.gpsimd.collective_compute(
                    "AllGather", mybir.AluOpType.bypass, replica_groups=RG,
                    ins=[bin_[:].opt()], outs=[bout[:].opt()],
                )
                # keep the PE clock warm through the collective window
                wprev = warm0
                for li in range(3):
                    wt = wp.tile([128, 2048], BF16, name=f"warmt", tag="warmt")
                    nc.gpsimd.tensor_copy(wt[:], wprev[:])
                    wps = tp.tile([2, 2], F32, name="wps", tag="wps")
                    nc.tensor.matmul(out=wps[:], lhsT=wt[:, 0:2], rhs=wt[:, 0:2],
                                     start=True, stop=True)
                    wprev = wt
                g = wp.tile([128, NC, PW], BF16, name="g", tag="g")
                gsrc = bout[:].rearrange("(kk p) f -> p kk f", kk=NC)
                nc.sync.dma_start(out=g[:, 0:4, :], in_=gsrc[:, 0:4, :])
                nc.scalar.dma_start(out=g[:, 4:8, :], in_=gsrc[:, 4:8, :])

                # ---- softmax weights -> diag halves ----
                scores = wp.tile([128, P16], F32, name="scores", tag="scores")
                nc.vector.tensor_reduce(
                    scores[:],
                    g[:, :, 0:P16].rearrange("n kk q -> n q kk"),
                    mybir.AxisListType.X, mybir.AluOpType.add,
                )
                e_b = wp.tile([128, P16], BF16, name="e_b", tag="e_b")
                den = wp.tile([128, 1], F32, name="den", tag="den")
                nc.scalar.activation(
                    out=e_b[:], in_=scores[:], func=mybir.ActivationFunctionType.Exp,
                    scale=1.0 / 32.0, accum_out=den[:],
                )
                rden = wp.tile([128, 1], F32, name="rden", tag="rden")
                nc.vector.reciprocal(rden[:], den[:])
                diag = wp.tile([128, P16, 128], BF16, name="diag", tag="diag")
                for half in range(2):
                    lo, hi = half * 8, half * 8 + 8
                    nc.vector.scalar_tensor_tensor(
                        out=diag[:, lo:hi, :],
                        in0=ident_rep[:, lo:hi, :],
                        scalar=rden[:],
                        in1=e_b[:, lo:hi].unsqueeze(2).broadcast_to([128, 8, 128]),
                        op0=mybir.AluOpType.mult,
                        op1=mybir.AluOpType.mult,
                    )

                # ---- act += h @ Wh + sum_p w_p * B_p ----
                act = act_prev
                for kk in range(KT):
                    nc.tensor.matmul(
                        out=act[:], lhsT=g[:, kk, P16:], rhs=wh_b[:, kk, :],
                        start=False, stop=False,
                    )
                for p in range(P16):
                    nc.tensor.matmul(
                        out=act[:], lhsT=diag[:, p, :], rhs=B_s[:, p, :],
                        start=False, stop=(p == P16 - 1),
                    )

                # prefetch next step's x@Wx while gates run
                if t + 1 < T:
                    act_prev = xwx_prefetch(t + 1)

                # ---- gates, state update ----
                sig = wp.tile([128, 3 * HS], F32, name="sig", tag="sig")
                nc.scalar.activation(
                    out=sig[:], in_=act[:, 0 : 3 * HS],
                    func=mybir.ActivationFunctionType.Sigmoid,
                )
                gg = wp.tile([128, HS], F32, name="gg", tag="gg")
                nc.scalar.activation(
                    out=gg[:], in_=act[:, 3 * HS : JS],
                    func=mybir.ActivationFunctionType.Tanh,
                )
                fc = wp.tile([128, HS], F32, name="fc", tag="fc")
                nc.vector.tensor_mul(fc[:], sig[:, HS : 2 * HS], c_cur[:])
                ig = wp.tile([128, HS], F32, name="ig", tag="ig")
                nc.vector.tensor_mul(ig[:], sig[:, 0:HS], gg[:])
                c_new = sp.tile([128, HS], F32, name="c", tag="c")
                nc.vector.tensor_add(c_new[:], fc[:], ig[:])
                tc_t = wp.tile([128, HS], F32, name="tc_t", tag="tc_t")
                nc.scalar.activation(
                    out=tc_t[:], in_=c_new[:], func=mybir.ActivationFunctionType.Tanh,
                )
                h_new = sp.tile([128, HS], BF16, name="h", tag="h")
                nc.vector.tensor_mul(h_new[:], sig[:, 2 * HS : 3 * HS], tc_t[:])
                nc.sync.dma_start(out=out_d[t], in_=h_new[:])
                h_cur, c_cur = h_new, c_new

    nc.compile()
    return nc


def _get_nc():
    global _nc_cache
    if _nc_cache is None:
        _nc_cache = _build()
    return _nc_cache


def _prepare_in_maps(x, A, Wx, Wh, Wattn, b):
    x = np.asarray(x, dtype=np.float32)
    A = np.asarray(A, dtype=np.float32)
    Wx = np.asarray(Wx, dtype=np.float32)
    Wh = np.asarray(Wh, dtype=np.float32)
    Wattn = np.asarray(Wattn, dtype=np.float32)
    b = np.asarray(b, dtype=np.float32)

    xT = np.ascontiguousarray(
        x.transpose(1, 2, 0).reshape(T, KT, 128, N).transpose(0, 2, 1, 3)
    )  # (T, 128, KT, N): partition-major for 4KB DMA rows
    Af = A.reshape(N, H, P16)
    afT = np.ascontiguousarray(Af.transpose(1, 2, 0))        # (H, P16, N)
    ident = np.eye(128, dtype=np.float32)

    def cols(W, k):
        return np.ascontiguousarray(
            np.concatenate([W[:, g * H + k * HS : g * H + (k + 1) * HS] for g in range(4)], axis=1)
        )

    in_maps = []
    for k in range(NC):
        afn = np.ascontiguousarray(
            Af[:, k * HS : (k + 1) * HS, :].transpose(0, 2, 1)  # (N, P16, HS)
        )
        bk = np.concatenate([b[g * H + k * HS : g * H + (k + 1) * HS] for g in range(4)])
        in_maps.append({
            "xT": xT,
            "wx": cols(Wx, k),
            "wh": cols(Wh, k),
            "wattn": cols(Wattn, k),
            "bvec": bk.reshape(1, JS),
            "ident": ident,
            "afn": afn,
            "afT": afT,
        })
    return in_maps


def _assemble(results):
    # per-core out: (T, N, HS) -> full (N, T, H)
    full = np.empty((N, T, H), dtype=np.float32)
    for k in range(NC):
        full[:, :, k * HS : (k + 1) * HS] = np.asarray(
            results[k]["out"], dtype=np.float32
        ).transpose(1, 0, 2)
    return full


def kernel(**inputs) -> np.ndarray:
    nc = _get_nc()
    in_maps = _prepare_in_maps(**inputs)
    res = run_bass_kernel_spmd(nc, in_maps, core_ids=list(range(NC)))
    return _assemble(res.results)


# revision 8
# speedup vs baseline: 1.4932x; 1.4932x over previous
"""AttentionLSTM Trainium2 kernel, 8-core SPMD.

Strategy: tensor-parallel over the 4H gate dimension. Core k owns H-slice
hd in [128k, 128(k+1)) of each of the four gates (512 act columns, laid out
[i_k | f_k | o_k | g_k]) and the matching slices of the c/h state.

Per timestep, one AllGather carries [partial_scores(16) | hT-slice(128)]
per batch row (bf16): the gathered hT blocks form exactly the lhsT k-tiles
for h@Wh, and the 8 partial-score blocks sum to the full attention scores
on every core.  The attention's contribution to the gate pre-activations
is computed without materializing attn: the prologue builds
B[n,p,j] = sum_h Af[n,h,p] * Wattn[h,j]  (j-sharded), and each step does
act += sum_p w[n,p] * B[n,p,j] as 16 diagonal matmuls diag(w_p) @ B_p
accumulated into the same PSUM bank as x@Wx (prefetched during the
AllGather window) and h@Wh.
"""
import numpy as np

from concourse import bacc, tile
from concourse import mybir
from concourse.bass_utils import run_bass_kernel_spmd

N, T, D, H = 128, 64, 1024, 1024
P16 = 16          # attention positions (4x4)
NC = 8            # cores
HS = H // NC      # 128, per-core H slice
JS = 4 * HS       # 512, per-core act columns
KT = D // 128     # 8 k-tiles

F32 = mybir.dt.float32
F32R = mybir.dt.float32r
BF16 = mybir.dt.bfloat16
RG = [list(range(NC))]

_nc_cache = None


def _build():
    nc = bacc.Bacc("TRN2", target_bir_lowering=False, debug=False, num_devices=NC)

    xT_d = nc.dram_tensor("xT", [T, 128, KT, N], F32, kind="ExternalInput").ap()
    wx_d = nc.dram_tensor("wx", [D, JS], F32, kind="ExternalInput").ap()
    wh_d = nc.dram_tensor("wh", [H, JS], F32, kind="ExternalInput").ap()
    wattn_d = nc.dram_tensor("wattn", [H, JS], F32, kind="ExternalInput").ap()
    b_d = nc.dram_tensor("bvec", [1, JS], F32, kind="ExternalInput").ap()
    ident_d = nc.dram_tensor("ident", [128, 128], F32, kind="ExternalInput").ap()
    afn_d = nc.dram_tensor("afn", [N, P16, HS], F32, kind="ExternalInput").ap()
    afT_d = nc.dram_tensor("afT", [H, P16, N], F32, kind="ExternalInput").ap()
    out_d = nc.dram_tensor("out", [T, N, HS], BF16, kind="ExternalOutput").ap()

    with tile.TileContext(nc) as tc:
        with (
            tc.tile_pool(name="const", bufs=1) as cp,
            tc.tile_pool(name="state", bufs=2) as sp,
            tc.tile_pool(name="work", bufs=2) as wp,
            tc.tile_pool(name="xpool", bufs=3) as xp,
            tc.tile_pool(name="psum", bufs=2, space="PSUM") as pp,
            tc.tile_pool(name="tpsum", bufs=2, space="PSUM") as tp,
            tc.tile_pool(name="dram", bufs=2, space="DRAM") as dp,
        ):
            # ---------------- constants ----------------
            wx_b = cp.tile([128, KT, JS], BF16, name="wx_b")
            b_b = cp.tile([1, JS], BF16, name="b_b")
            ones_b = cp.tile([1, 128], BF16, name="ones_b")
            nc.vector.memset(ones_b[:], 1.0)
            ident = cp.tile([128, 128], F32, name="ident")
            nc.sync.dma_start(out=ident[:], in_=ident_d[:])
            ident_b = cp.tile([128, 128], BF16, name="ident_b")
            nc.vector.tensor_copy(ident_b[:], ident[:])
            ident_rep = cp.tile([128, P16, 128], BF16, name="ident_rep")
            nc.vector.tensor_copy(
                ident_rep[:], ident_b[:].unsqueeze(1).broadcast_to([128, P16, 128])
            )
            wh_b = cp.tile([128, KT, JS], BF16, name="wh_b")
            afn_b = cp.tile([128, P16, HS], BF16, name="afn_b")
            B_s = cp.tile([128, P16, JS], BF16, name="B_s")

            # ---------------- prologue ----------------
            h_cur = sp.tile([128, HS], BF16, name="h", tag="h")
            c_cur = sp.tile([128, HS], F32, name="c", tag="c")
            with (
                tc.tile_pool(name="prol", bufs=1) as pr,
                tc.tile_pool(name="prolp", bufs=2, space="PSUM") as prp,
            ):
                wat_b = pr.tile([128, KT, JS], BF16, name="wat_b")
                afT_b = pr.tile([128, KT, P16, N], BF16, name="afT_b")
                for wsrc, wdst in ((wx_d, wx_b), (wh_d, wh_b), (wattn_d, wat_b)):
                    wst = pr.tile([128, KT, JS], F32, name="wst", tag="wst", bufs=2)
                    nc.sync.dma_start(out=wst[:], in_=wsrc[:].rearrange("(kk p) j -> p kk j", kk=KT))
                    nc.vector.tensor_copy(wdst[:], wst[:])
                b_st = pr.tile([1, JS], F32, name="b_st")
                nc.sync.dma_start(out=b_st[:], in_=b_d[:])
                nc.vector.tensor_copy(b_b[:], b_st[:])

                afn_st = pr.tile([128, P16, HS], F32, name="afn_st")
                nc.sync.dma_start(out=afn_st[:], in_=afn_d[:])
                nc.vector.tensor_copy(afn_b[:], afn_st[:])
                # h0 = mean over p of Af (slice), c0 = h0
                h0r = pr.tile([128, HS], F32, name="h0r")
                nc.vector.tensor_reduce(
                    h0r[:], afn_st[:].rearrange("n p hd -> n hd p"),
                    mybir.AxisListType.X, mybir.AluOpType.add,
                )
                nc.scalar.mul(out=h_cur[:], in_=h0r[:], mul=1.0 / P16)
                nc.scalar.mul(out=c_cur[:], in_=h0r[:], mul=1.0 / P16)

                # B[n,p,j] = sum_h Af[n,h,p] Wattn[h,j]
                afT_src = afT_d[:].rearrange("(kk p) q n -> p kk q n", kk=KT)
                for kk in range(KT):
                    afT_st = pr.tile([128, P16, N], F32, name="afT_st", tag="afT_st", bufs=2)
                    nc.sync.dma_start(out=afT_st[:], in_=afT_src[:, kk])
                    nc.vector.tensor_copy(afT_b[:, kk], afT_st[:])
                for p in range(P16):
                    bp = prp.tile([128, JS], F32, name="bp", tag="bp")
                    for kk in range(KT):
                        nc.tensor.matmul(
                            out=bp[:],
                            lhsT=afT_b[:, kk, p, :],
                            rhs=wat_b[:, kk, :],
                            start=(kk == 0), stop=(kk == KT - 1),
                        )
                    nc.scalar.copy(out=B_s[:, p, :], in_=bp[:])

            # ---------------- recurrence ----------------
            act_prev = None

            def xwx_prefetch(t):
                """x_t @ Wx + b into a fresh act psum bank (during AG window)."""
                xt = xp.tile([128, KT, N], F32, name="xt", tag="xt")
                nc.sync.dma_start(out=xt[:], in_=xT_d[t])
                xt_b = xp.tile([128, KT, N], BF16, name="xt_b", tag="xt_b")
                nc.scalar.copy(out=xt_b[:], in_=xt[:])
                act = pp.tile([128, JS], F32, name="act", tag="act")
                for kk in range(KT):
                    nc.tensor.matmul(
                        out=act[:],
                        lhsT=xt_b[:, kk, :],
                        rhs=wx_b[:, kk, :],
                        start=(kk == 0), stop=False,
                    )
                nc.tensor.matmul(
                    out=act[:], lhsT=ones_b[:], rhs=b_b[:],
                    start=False, stop=False,
                )
                return act

            act_prev = xwx_prefetch(0)

            for t in range(T):
                PW = 2 * P16 + 128  # payload bf16 cols: 16 f32-bit pscores + 128 hT
                # ---- payload: [pscores bf16 | hT] in one SBUF tile ----
                bin_ = dp.tile([128, PW], BF16, name="bin", tag="bin")
                bout = dp.tile([NC * 128, PW], BF16, addr_space="Shared",
                               name="bout", tag="bout")
                payload = wp.tile([128, PW], BF16, name="payload", tag="payload")
                hTp = tp.tile([128, 128], BF16, name="hTp", tag="hTp")
                nc.tensor.transpose(hTp[:], h_cur[:], ident_b[:])
                nc.scalar.copy(out=payload[:, 2 * P16 :], in_=hTp[:])
                prod = wp.tile([128, P16, HS], BF16, name="prod", tag="prod")
                nc.vector.tensor_mul(
                    prod[:], afn_b[:],
                    h_cur[:].unsqueeze(1).broadcast_to([128, P16, HS]),
                )
                pfold = wp.tile([128, P16, HS // 2], BF16, name="pfold", tag="pfold")
                nc.vector.tensor_add(
                    pfold[:], prod[:, :, 0 : HS // 2], prod[:, :, HS // 2 :]
                )
                nc.vector.tensor_reduce(
                    payload[:, 0 : 2 * P16].bitcast(F32), pfold[:],
                    mybir.AxisListType.X, mybir.AluOpType.add,
                )
                nc.sync.dma_start(out=bin_[0:64, :], in_=payload[0:64, :])
                nc.scalar.dma_start(out=bin_[64:128, :], in_=payload[64:128, :])

                # ---- AllGather + gather readback split across both queues ----
                nc.gpsimd.collective_compute(
                    "AllGather", mybir.AluOpType.bypass, replica_groups=RG,
                    ins=[bin_[:].opt()], outs=[bout[:].opt()],
                )
                g = wp.tile([128, NC, PW], BF16, name="g", tag="g")
                gsrc = bout[:].rearrange("(kk p) f -> p kk f", kk=NC)
                nc.sync.dma_start(out=g[:, 0:4, :], in_=gsrc[:, 0:4, :])
                nc.scalar.dma_start(out=g[:, 4:8, :], in_=gsrc[:, 4:8, :])

                # ---- softmax weights -> diag halves ----
                scores = wp.tile([128, P16], F32, name="scores", tag="scores")
                nc.vector.tensor_reduce(
                    scores[:],
                    g[:, :, 0 : 2 * P16].bitcast(F32).rearrange("n kk q -> n q kk"),
                    mybir.AxisListType.X, mybir.AluOpType.add,
                )
                e_b = wp.tile([128, P16], BF16, name="e_b", tag="e_b")
                den = wp.tile([128, 1], F32, name="den", tag="den")
                nc.scalar.activation(
                    out=e_b[:], in_=scores[:], func=mybir.ActivationFunctionType.Exp,
                    scale=1.0 / 32.0, accum_out=den[:],
                )
                rden = wp.tile([128, 1], F32, name="rden", tag="rden")
                nc.vector.reciprocal(rden[:], den[:])
                diag = wp.tile([128, P16, 128], BF16, name="diag", tag="diag")
                for half in range(2):
                    lo, hi = half * 8, half * 8 + 8
                    nc.vector.scalar_tensor_tensor(
                        out=diag[:, lo:hi, :],
                        in0=ident_rep[:, lo:hi, :],
                        scalar=rden[:],
                        in1=e_b[:, lo:hi].unsqueeze(2).broadcast_to([128, 8, 128]),
                        op0=mybir.AluOpType.mult,
                        op1=mybir.AluOpType.mult,
                    )

                # ---- act += h @ Wh + sum_p w_p * B_p ----
                act = act_prev
                for kk in range(KT):
                    nc.tensor.matmul(
                        out=act[:], lhsT=g[:, kk, 2 * P16 :], rhs=wh_b[:, kk, :],
                        start=False, stop=False,
                    )
                for p in range(P16):
                    nc.tensor.matmul(
                        out=act[:], lhsT=diag[:, p, :], rhs=B_s[:, p, :],
                        start=False, stop=(p == P16 - 1),
                    )

                # prefetch next step's x@Wx while gates run
                if t + 1 < T:
                    act_prev = xwx_prefetch(t + 1)

                # ---- gates, state update ----
                sig = wp.tile([128, 3 * HS], F32, name="sig", tag="sig")
                nc.scalar.activation(
                    out=sig[:], in_=act[:, 0 : 3 * HS],
                    func=mybir.ActivationFunctionType.Sigmoid,
                )
                gg = wp.tile([128, HS], F32, name="gg", tag="gg")
                nc.scalar.activation(
                    out=gg[:], in_=act[:, 3 * HS : JS],
                    func=mybir.ActivationFunctionType.Tanh,
                )
                fc = wp.tile([128, HS], F32, name="fc", tag="fc")
                nc.vector.tensor_mul(fc[:], sig[:, HS : 2 * HS], c_cur[:])
                ig = wp.tile([128, HS], F32, name="ig", tag="ig")
                nc.vector.tensor_mul(ig[:], sig[:, 0:HS], gg[:])
                c_new = sp.tile([128, HS], F32, name="c", tag="c")
                nc.vector.tensor_add(c_new[:], fc[:], ig[:])
                tc_t = wp.tile([128, HS], F32, name="tc_t", tag="tc_t")
                nc.scalar.activation(
                    out=tc_t[:], in_=c_new[:], func=mybir.ActivationFunctionType.Tanh,
                )
                h_new = sp.tile([128, HS], BF16, name="h", tag="h")
                nc.vector.tensor_mul(h_new[:], sig[:, 2 * HS : 3 * HS], tc_t[:])
                nc.sync.dma_start(out=out_d[t], in_=h_new[:])
                h_cur, c_cur = h_new, c_new

    nc.compile()
    return nc


def _get_nc():
    global _nc_cache
    if _nc_cache is None:
        _nc_cache = _build()
    return _nc_cache


def _prepare_in_maps(x, A, Wx, Wh, Wattn, b):
    x = np.asarray(x, dtype=np.float32)
    A = np.asarray(A, dtype=np.float32)
    Wx = np.asarray(Wx, dtype=np.float32)
    Wh = np.asarray(Wh, dtype=np.float32)
    Wattn = np.asarray(Wattn, dtype=np.float32)
    b = np.asarray(b, dtype=np.float32)

    xT = np.ascontiguousarray(
        x.transpose(1, 2, 0).reshape(T, KT, 128, N).transpose(0, 2, 1, 3)
    )  # (T, 128, KT, N): partition-major for 4KB DMA rows
    Af = A.reshape(N, H, P16)
    afT = np.ascontiguousarray(Af.transpose(1, 2, 0))        # (H, P16, N)
    ident = np.eye(128, dtype=np.float32)

    def cols(W, k):
        return np.ascontiguousarray(
            np.concatenate([W[:, g * H + k * HS : g * H + (k + 1) * HS] for g in range(4)], axis=1)
        )

    in_maps = []
    for k in range(NC):
        afn = np.ascontiguousarray(
            Af[:, k * HS : (k + 1) * HS, :].transpose(0, 2, 1)  # (N, P16, HS)
        )
        bk = np.concatenate([b[g * H + k * HS : g * H + (k + 1) * HS] for g in range(4)])
        in_maps.append({
            "xT": xT,
            "wx": cols(Wx, k),
            "wh": cols(Wh, k),
            "wattn": cols(Wattn, k),
            "bvec": bk.reshape(1, JS),
            "ident": ident,
            "afn": afn,
            "afT": afT,
        })
    return in_maps


def _assemble(results):
    # per-core out: (T, N, HS) -> full (N, T, H)
    full = np.empty((N, T, H), dtype=np.float32)
    for k in range(NC):
        full[:, :, k * HS : (k + 1) * HS] = np.asarray(
            results[k]["out"], dtype=np.float32
        ).transpose(1, 0, 2)
    return full


def kernel(**inputs) -> np.ndarray:
    nc = _get_nc()
    in_maps = _prepare_in_maps(**inputs)
    res = run_bass_kernel_spmd(nc, in_maps, core_ids=list(range(NC)))
    return _assemble(res.results)
